# revision 7
# baseline (speedup 1.0000x reference)
"""Distributed Trainium2 Bass kernel for the A2C GNN message-passing model.

Strategy (8 NeuronCores, node-row sharding, 512 rows/core):
  - SAGE aggregation as dense-adjacency matmuls on TensorE: the host builds
    AT[j,i] = count(src=j -> dst=i) / max(indeg(i),1) once; each core keeps its
    512-column slice in SBUF (fp16) and computes agg_T[f, own_i] = sum_k
    h_nat[k-chunk] @ AT[k-chunk] (32 accumulating matmuls, f32 PSUM).
  - Feature maps are kept transposed ([feat, node]) so SAGE biases are
    per-partition activation biases; tanh fused into the PSUM->SBUF copy.
  - After layers 1 and 2 the 512 new rows are AllGathered (fp16, 128KB/rank)
    to rebuild the full natural-layout h for the next aggregation.
  - Actor and critic heads share the layer-3 aggregation. Only the projected
    scalars a = Xa@w1, b = Xa@w2, cp = Xc@wfc are AllGathered (f32, 6KB).
  - The N^2 log_softmax factorizes: out[i,j] = a_i + b_j + bfa - LSE with
    LSE = bfa + logsumexp(a) + logsumexp(b), so bfa cancels and
    out[i,j] = b_j + (a_i - La - Lb). Each core writes its 512x4096 f32 block:
    partition-broadcast of the b row + one tensor_scalar add per tile.
  - edge_critic = tanh(mean(cp) + bfc) (the (cp_i+cp_j)/2 mean collapses).

Perf notes (trace-driven):
  - A dummy AllGather with no dependencies runs first so the model-entry CC
    barrier / stream spin-up / cold RDH overlap the HBM load phase instead of
    stalling the first real collective.
  - amat/xnat are staged in DRAM partition-major so each loads as ONE
    dma_start (dispatch on the Sync/Act queues costs ~0.4-2.4us per DMA).
  - The post-AllGather h reload is split per rank-block so layer k+1's
    accumulation matmuls start while later blocks are still in flight.
  - The output is built in 16 [128,1024] tiles: b-row broadcast quarters on
    GpSimd pipeline with DVE adds and output DMAs alternating Sync/Act.
"""

import numpy as np

N = 4096
U = 128
NCORES = 8
R = N // NCORES          # 512 rows per core
KT = N // 128            # 32 contraction chunks
RC = R // 128            # 4 row chunks per core

fp16 = np.float16

_STATE = {}


def _build_nc():
    import concourse.bass as bass
    import concourse.bacc as bacc
    import concourse.mybir as mybir
    import concourse.tile as tile

    f32 = mybir.dt.float32
    f16 = mybir.dt.float16
    AX = mybir.AxisListType.X
    AF = mybir.ActivationFunctionType

    nc = bacc.Bacc("TRN2", target_bir_lowering=False, debug=False,
                   num_devices=NCORES)

    # ---- kernel I/O ----
    amat = nc.dram_tensor("amat", [128, KT * R], f16, kind="ExternalInput")
    xnat = nc.dram_tensor("xnat", [128, KT * U], f16, kind="ExternalInput")
    xt = nc.dram_tensor("xt", [U, R], f16, kind="ExternalInput")
    wts = nc.dram_tensor("wts", [U, 8 * U], f16, kind="ExternalInput")
    biases = nc.dram_tensor("biases", [U, 4], f32, kind="ExternalInput")
    w12 = nc.dram_tensor("w12", [U, 2], f16, kind="ExternalInput")
    wfc = nc.dram_tensor("wfc", [U, 1], f16, kind="ExternalInput")
    bfc = nc.dram_tensor("bfc", [1, 1], f32, kind="ExternalInput")
    iden = nc.dram_tensor("iden", [128, 128], f16, kind="ExternalInput")
    ones1 = nc.dram_tensor("ones1", [1, 1], f32, kind="ExternalInput")
    ones128 = nc.dram_tensor("ones128", [128, 1], f32, kind="ExternalInput")

    out = nc.dram_tensor("out", [R, N], f32, kind="ExternalOutput")
    crit = nc.dram_tensor("crit", [1, 1], f32, kind="ExternalOutput")

    # ---- collective bounce buffers (internal DRAM) ----
    wu_in = nc.dram_tensor("wu_in", [RC, 128, U], f16)
    wu_out = nc.dram_tensor("wu_out", [KT, 128, U], f16, addr_space="Shared")
    ag_in = [nc.dram_tensor(f"ag{l}_in", [RC, 128, U], f16) for l in (1, 2)]
    ag_out = [nc.dram_tensor(f"ag{l}_out", [NCORES, RC, 128, U], f16,
                             addr_space="Shared") for l in (1, 2)]
    ag3_in = nc.dram_tensor("ag3_in", [3, R], f32)
    ag3_out = nc.dram_tensor("ag3_out", [NCORES, 3, R], f32,
                             addr_space="Shared")
    rgroups = [list(range(NCORES))]

    with tile.TileContext(nc) as tc:
        with tc.tile_pool(name="const", bufs=1) as cst, \
             tc.tile_pool(name="work", bufs=1) as wrk, \
             tc.tile_pool(name="big", bufs=1) as big, \
             tc.tile_pool(name="opool", bufs=4) as opool, \
             tc.tile_pool(name="pag", bufs=1, space="PSUM") as pagp, \
             tc.tile_pool(name="pw", bufs=1, space="PSUM") as pwp, \
             tc.tile_pool(name="pt", bufs=1, space="PSUM") as ptp, \
             tc.tile_pool(name="psm", bufs=2, space="PSUM") as psm:

            # warmup collective: absorbs the entry barrier + CC stream
            # spin-up + cold RDH concurrently with the HBM loads below.
            zw = wrk.tile([128, RC * U], f16, tag="zw")
            nc.vector.memset(zw[:], 0.0)
            nc.sync.dma_start(wu_in.ap(), zw[:])
            nc.gpsimd.collective_compute(
                "AllGather", mybir.AluOpType.bypass,
                ins=[wu_in.ap().opt()], outs=[wu_out.ap().opt()],
                replica_groups=rgroups)

            # ---- constant loads (few big dispatches, split across queues) --
            xnat_sb = cst.tile([128, KT * U], f16, tag="xnat")
            nc.scalar.dma_start(xnat_sb[:], xnat[:, :])
            amat_sb = cst.tile([128, KT * R], f16, tag="amat")
            nc.sync.dma_start(amat_sb[:, :KT * R // 2], amat[:, :KT * R // 2])
            nc.sync.dma_start(amat_sb[:, KT * R // 2:], amat[:, KT * R // 2:])
            xt_sb = cst.tile([128, R], f16, tag="xt")
            nc.scalar.dma_start(xt_sb[:], xt[:, :])
            wts_sb = cst.tile([128, 8 * U], f16, tag="wts")
            nc.scalar.dma_start(wts_sb[:], wts[:, :])
            bias_sb = cst.tile([128, 4], f32, tag="bias")
            nc.scalar.dma_start(bias_sb[:], biases[:, :])
            w12_sb = cst.tile([128, 2], f16, tag="w12")
            nc.scalar.dma_start(w12_sb[:], w12[:, :])
            wfc_sb = cst.tile([128, 1], f16, tag="wfc")
            nc.scalar.dma_start(wfc_sb[:], wfc[:, :])
            bfc_sb = cst.tile([1, 1], f32, tag="bfc")
            nc.scalar.dma_start(bfc_sb[:], bfc[:, :])
            iden_sb = cst.tile([128, 128], f16, tag="iden")
            nc.scalar.dma_start(iden_sb[:], iden[:, :])
            one1_sb = cst.tile([1, 1], f32, tag="one1")
            nc.scalar.dma_start(one1_sb[:], ones1[:, :])
            one128_sb = cst.tile([128, 1], f32, tag="one128")
            nc.scalar.dma_start(one128_sb[:], ones128[:, :])

            hnat = [None, None]   # full natural h (layers 1, 2)
            hT = [None, None]     # transposed own-columns h

            def sage_agg(lhs_sb):
                """agg_T[f, own_i] accumulated over 32 k-chunk matmuls."""
                pag = pagp.tile([128, R], f32, tag="pag")
                for k in range(KT):
                    nc.tensor.matmul(pag[:], lhs_sb[:, k * U:(k + 1) * U],
                                     amat_sb[:, k * R:(k + 1) * R],
                                     start=(k == 0), stop=(k == KT - 1))
                aggT = wrk.tile([128, R], f16, tag="aggT")
                nc.scalar.copy(aggT[:], pag[:])
                return aggT

            # ================= layers 1 and 2 =================
            for l in range(2):
                lhs = xnat_sb if l == 0 else hnat[0]
                rhsT = xt_sb if l == 0 else hT[0]
                aggT = sage_agg(lhs)
                ph = pwp.tile([128, R], f32, tag="ph")
                nc.tensor.matmul(ph[:], wts_sb[:, (2 * l) * U:(2 * l + 1) * U],
                                 aggT[:], start=True, stop=False)
                nc.tensor.matmul(ph[:], wts_sb[:, (2 * l + 1) * U:(2 * l + 2) * U],
                                 rhsT[:], start=False, stop=True)
                hT_new = wrk.tile([128, R], f16, tag=f"hT{l}")
                nc.scalar.activation(hT_new[:], ph[:], AF.Tanh,
                                     bias=bias_sb[:, l:l + 1])
                hT[l] = hT_new
                # transpose own columns back to natural layout
                pt = ptp.tile([128, R], f16, tag="pt")
                for c in range(RC):
                    nc.tensor.transpose(pt[:, c * 128:(c + 1) * 128],
                                        hT_new[:, c * 128:(c + 1) * 128],
                                        iden_sb[:])
                hc = wrk.tile([128, R], f16, tag=f"hc{l}")
                nc.scalar.copy(hc[:], pt[:])
                nc.sync.dma_start(
                    ag_in[l].ap().rearrange("c p f -> p c f"),
                    hc[:].rearrange("p (c f) -> p c f", c=RC))
                nc.gpsimd.collective_compute(
                    "AllGather", mybir.AluOpType.bypass,
                    ins=[ag_in[l].ap().opt()], outs=[ag_out[l].ap().opt()],
                    replica_groups=rgroups)
                hn = big.tile([128, KT * U], f16, tag=f"hnat{l}")
                for r in range(NCORES):
                    eng = nc.sync if r % 2 == 0 else nc.scalar
                    eng.dma_start(
                        hn[:, r * RC * U:(r + 1) * RC * U].rearrange(
                            "p (k f) -> p k f", k=RC),
                        ag_out[l][r].rearrange("k p f -> p k f"))
                hnat[l] = hn

            # ================= heads (shared aggregation) =================
            aggT3 = sage_agg(hnat[1])
            headT = []
            for hi, (wl, wr, bcol) in enumerate(((4, 5, 2), (6, 7, 3))):
                ph = pwp.tile([128, R], f32, tag="ph")
                nc.tensor.matmul(ph[:], wts_sb[:, wl * U:(wl + 1) * U],
                                 aggT3[:], start=True, stop=False)
                nc.tensor.matmul(ph[:], wts_sb[:, wr * U:(wr + 1) * U],
                                 hT[1][:], start=False, stop=True)
                xh = wrk.tile([128, R], f16, tag=f"headT{hi}")
                nc.vector.tensor_scalar_add(xh[:], ph[:], bias_sb[:, bcol:bcol + 1])
                headT.append(xh)

            pab = psm.tile([2, R], f32, tag="small")
            nc.tensor.matmul(pab[:], w12_sb[:], headT[0][:], start=True, stop=True)
            pcp = psm.tile([1, R], f32, tag="small")
            nc.tensor.matmul(pcp[:], wfc_sb[:], headT[1][:], start=True, stop=True)
            abc_own = wrk.tile([2, R], f32, tag="abc_own")
            nc.scalar.copy(abc_own[:], pab[:])
            cp_own = wrk.tile([1, R], f32, tag="cp_own")
            nc.scalar.copy(cp_own[:], pcp[:])
            nc.sync.dma_start(ag3_in[0:2, :], abc_own[:])
            nc.sync.dma_start(ag3_in[2, :], cp_own[:])
            nc.gpsimd.collective_compute(
                "AllGather", mybir.AluOpType.bypass,
                ins=[ag3_in.ap().opt()], outs=[ag3_out.ap().opt()],
                replica_groups=rgroups)

            # ---- global logsumexp of a and b (parallel [128, 32] layout) ----
            abcpm = wrk.tile([128, 3 * 32], f32, tag="abcpm")
            for t in range(3):
                nc.sync.dma_start(abcpm[:, t * 32:(t + 1) * 32],
                                  ag3_out[:, t, :])
            b_row = wrk.tile([1, N], f32, tag="b_row")
            nc.scalar.dma_start(b_row[:], ag3_out[:, 1, :])

            Ls = []
            for t in range(2):
                v = abcpm[:, t * 32:(t + 1) * 32]
                negm = wrk.tile([128, 1], f32, tag=f"negm{t}")
                nc.vector.reduce_max(negm[:], v, axis=AX, negate=True)
                e = wrk.tile([128, 32], f32, tag=f"e{t}")
                es = wrk.tile([128, 1], f32, tag=f"es{t}")
                nc.scalar.activation(e[:], v, AF.Exp, bias=negm[:, 0:1],
                                     accum_out=es[:, 0:1])
                emp = wrk.tile([128, 1], f32, tag=f"emp{t}")
                nc.scalar.activation(emp[:], negm[:], AF.Exp, scale=-1.0)
                # total = sum_p es_p * exp(m_p) as a PE dot product
                ptot = psm.tile([1, 1], f32, tag="small")
                nc.tensor.matmul(ptot[:], es[:], emp[:], start=True, stop=True)
                L = wrk.tile([1, 1], f32, tag=f"L{t}")
                nc.scalar.activation(L[:], ptot[:], AF.Ln)
                Ls.append(L)

            negL2 = wrk.tile([1, 1], f32, tag="negL2")
            nc.vector.tensor_tensor(negL2[:], Ls[0][:], Ls[1][:],
                                    op=mybir.AluOpType.add)
            nc.scalar.mul(negL2[:], negL2[:], -1.0)

            # critic: tanh(sum(cp)/N + bfc)
            cps = wrk.tile([128, 1], f32, tag="cps")
            nc.vector.reduce_sum(cps[:], abcpm[:, 64:96], axis=AX)
            pct = psm.tile([1, 1], f32, tag="small")
            nc.tensor.matmul(pct[:], one128_sb[:], cps[:], start=True, stop=True)
            crit_sb = wrk.tile([1, 1], f32, tag="crit_sb")
            nc.scalar.activation(crit_sb[:], pct[:], AF.Tanh,
                                 scale=1.0 / N, bias=bfc_sb[0:1, 0:1])
            nc.sync.dma_start(crit[:, :], crit_sb[:])

            # alpha[i] = a_own[i] - La - Lb, moved to partition axis
            arow = wrk.tile([1, R], f32, tag="arow")
            nc.vector.tensor_scalar_add(arow[:], abc_own[0:1, :],
                                        negL2[0:1, 0:1])
            pa = psm.tile([128, RC], f32, tag="small")
            for c in range(RC):
                nc.tensor.matmul(pa[:, c:c + 1],
                                 arow[0:1, c * 128:(c + 1) * 128],
                                 one1_sb[:], start=True, stop=True)
            alpha = wrk.tile([128, RC], f32, tag="alpha")
            nc.scalar.copy(alpha[:], pa[:])

            # ---- the big output: out[i, j] = b[j] + alpha[i] ----
            # column-quartered pipeline: broadcast quarter -> 4 row tiles
            CQ = N // 4
            bb = big.tile([128, N], f32, tag="bb")
            di = 0
            for q in range(4):
                nc.gpsimd.partition_broadcast(bb[:, q * CQ:(q + 1) * CQ],
                                              b_row[0:1, q * CQ:(q + 1) * CQ])
                for c in range(RC):
                    ob = opool.tile([128, CQ], f32, tag="ob")
                    nc.vector.tensor_scalar_add(ob[:], bb[:, q * CQ:(q + 1) * CQ],
                                                alpha[:, c:c + 1])
                    eng = nc.sync if di % 2 == 0 else nc.scalar
                    di += 1
                    eng.dma_start(
                        out[c * 128:(c + 1) * 128, q * CQ:(q + 1) * CQ],
                        ob[:])

    nc.compile()
    return nc


def _get_nc():
    if "nc" not in _STATE:
        import concourse.bass as bass  # noqa: F401
        _STATE["nc"] = _build_nc()
    return _STATE["nc"]


def _host_prep(inputs):
    x = np.asarray(inputs["x"], np.float32)
    ei = np.asarray(inputs["edge_index"])
    src = ei[0].astype(np.int64)
    dst = ei[1].astype(np.int64)

    AT = np.zeros((N, N), np.float32)
    np.add.at(AT, (src, dst), 1.0)
    deg = np.bincount(dst, minlength=N).astype(np.float32)
    ATn = AT / np.maximum(deg, 1.0)[None, :]

    wts = np.concatenate([
        inputs["Wf_l"].T, inputs["Wf_r"].T,
        inputs["Wcm_l"].T, inputs["Wcm_r"].T,
        inputs["Wa_l"].T, inputs["Wa_r"].T,
        inputs["Wcr_l"].T, inputs["Wcr_r"].T,
    ], axis=1).astype(fp16)
    biases = np.stack([
        inputs["bf_l"], inputs["bcm_l"], inputs["ba_l"], inputs["bcr_l"],
    ], axis=1).astype(np.float32)
    Wfa = np.asarray(inputs["Wfa"], np.float32)
    w12 = np.stack([Wfa[0, :U], Wfa[0, U:]], axis=1).astype(fp16)
    wfc = np.asarray(inputs["Wfc"], np.float32)[0][:, None].astype(fp16)
    bfc = np.asarray(inputs["bfc"], np.float32).reshape(1, 1)

    common = {
        # partition-major: sbuf[p, k*U+f] = x[k*128+p, f]
        "xnat": np.ascontiguousarray(
            x.astype(fp16).reshape(KT, 128, U).transpose(1, 0, 2).reshape(
                128, KT * U)),
        "wts": wts,
        "biases": biases,
        "w12": w12,
        "wfc": wfc,
        "bfc": bfc,
        "iden": np.eye(128, dtype=fp16),
        "ones1": np.ones((1, 1), np.float32),
        "ones128": np.ones((128, 1), np.float32),
    }
    in_maps = []
    for c in range(NCORES):
        sl = slice(c * R, (c + 1) * R)
        m = dict(common)
        # partition-major: sbuf[p, k*R+i] = ATn[k*128+p, own_i]
        m["amat"] = np.ascontiguousarray(
            ATn[:, sl].astype(fp16).reshape(KT, 128, R).transpose(
                1, 0, 2).reshape(128, KT * R))
        m["xt"] = np.ascontiguousarray(x[sl].T.astype(fp16))
        in_maps.append(m)
    return in_maps


def _run(inputs, trace=False):
    from concourse.bass_utils import run_bass_kernel_spmd
    nc = _get_nc()
    in_maps = _host_prep(inputs)
    res = run_bass_kernel_spmd(nc, in_maps, core_ids=list(range(NCORES)),
                               trace=trace)
    edge_actor = np.concatenate(
        [np.asarray(res.results[c]["out"], np.float32) for c in range(NCORES)],
        axis=0).reshape(N * N, 1)
    edge_critic = np.asarray(res.results[0]["crit"], np.float32).reshape(1, 1)
    return (edge_actor, edge_critic), res


def kernel(**inputs):
    outputs, _ = _run(inputs, trace=False)
    return outputs


# revision 9
# speedup vs baseline: 1.0451x; 1.0451x over previous
"""Distributed Trainium2 Bass kernel for the A2C GNN message-passing model.

Strategy (8 NeuronCores, node-row sharding, 512 rows/core):
  - SAGE aggregation as dense-adjacency matmuls on TensorE: the host builds
    AT[j,i] = count(src=j -> dst=i) / max(indeg(i),1) once; each core keeps its
    512-column slice in SBUF (fp16) and computes agg_T[f, own_i] = sum_k
    h_nat[k-chunk] @ AT[k-chunk] (32 accumulating matmuls, f32 PSUM).
  - Feature maps are kept transposed ([feat, node]) so SAGE biases are
    per-partition activation biases; tanh fused into the PSUM->SBUF copy.
  - After layers 1 and 2 the 512 new rows are AllGathered (fp16, 128KB/rank)
    to rebuild the full natural-layout h for the next aggregation.
  - Actor and critic heads share the layer-3 aggregation. Only the projected
    scalars a = Xa@w1, b = Xa@w2, cp = Xc@wfc are AllGathered (f32, 6KB).
  - The N^2 log_softmax factorizes: out[i,j] = a_i + b_j + bfa - LSE with
    LSE = bfa + logsumexp(a) + logsumexp(b), so bfa cancels and
    out[i,j] = b_j + (a_i - La - Lb). Each core writes its 512x4096 f32 block:
    partition-broadcast of the b row + one tensor_scalar add per tile.
  - edge_critic = tanh(mean(cp) + bfc) (the (cp_i+cp_j)/2 mean collapses).

Perf notes (trace-driven):
  - A dummy AllGather with no dependencies runs first so the model-entry CC
    barrier / stream spin-up / cold RDH overlap the HBM load phase instead of
    stalling the first real collective.
  - amat/xnat are staged in DRAM partition-major so each loads as ONE
    dma_start (dispatch on the Sync/Act queues costs ~0.4-2.4us per DMA).
  - The post-AllGather h reload is split per rank-block so layer k+1's
    accumulation matmuls start while later blocks are still in flight.
  - The output is built in 16 [128,1024] tiles: b-row broadcast quarters on
    GpSimd pipeline with DVE adds and output DMAs alternating Sync/Act.
"""

import numpy as np

N = 4096
U = 128
NCORES = 8
R = N // NCORES          # 512 rows per core
KT = N // 128            # 32 contraction chunks
RC = R // 128            # 4 row chunks per core

fp16 = np.float16

_STATE = {}


def _build_nc():
    import concourse.bass as bass
    import concourse.bacc as bacc
    import concourse.mybir as mybir
    import concourse.tile as tile

    f32 = mybir.dt.float32
    f16 = mybir.dt.float16
    AX = mybir.AxisListType.X
    AF = mybir.ActivationFunctionType

    nc = bacc.Bacc("TRN2", target_bir_lowering=False, debug=False,
                   num_devices=NCORES)

    # ---- kernel I/O ----
    amat = nc.dram_tensor("amat", [128, KT * R], f16, kind="ExternalInput")
    xnat = nc.dram_tensor("xnat", [128, KT * U], f16, kind="ExternalInput")
    xt = nc.dram_tensor("xt", [U, R], f16, kind="ExternalInput")
    wts = nc.dram_tensor("wts", [U, 8 * U], f16, kind="ExternalInput")
    biases = nc.dram_tensor("biases", [U, 4], f32, kind="ExternalInput")
    w12 = nc.dram_tensor("w12", [U, 2], f16, kind="ExternalInput")
    wfc = nc.dram_tensor("wfc", [U, 1], f16, kind="ExternalInput")
    bfc = nc.dram_tensor("bfc", [1, 1], f32, kind="ExternalInput")
    iden = nc.dram_tensor("iden", [128, 128], f16, kind="ExternalInput")
    ones1 = nc.dram_tensor("ones1", [1, 1], f32, kind="ExternalInput")
    ones128 = nc.dram_tensor("ones128", [128, 1], f32, kind="ExternalInput")

    out = nc.dram_tensor("out", [R, N], f32, kind="ExternalOutput")
    crit = nc.dram_tensor("crit", [1, 1], f32, kind="ExternalOutput")

    # ---- collective bounce buffers (internal DRAM) ----
    ag_in = [nc.dram_tensor(f"ag{l}_in", [RC, 128, U], f16) for l in (1, 2)]
    ag_out = [nc.dram_tensor(f"ag{l}_out", [NCORES, RC, 128, U], f16,
                             addr_space="Shared") for l in (1, 2)]
    ag3_in = nc.dram_tensor("ag3_in", [3, R], f32)
    ag3_out = nc.dram_tensor("ag3_out", [NCORES, 3, R], f32,
                             addr_space="Shared")
    rgroups = [list(range(NCORES))]

    with tile.TileContext(nc) as tc:
        with tc.tile_pool(name="const", bufs=1) as cst, \
             tc.tile_pool(name="work", bufs=1) as wrk, \
             tc.tile_pool(name="big", bufs=1) as big, \
             tc.tile_pool(name="opool", bufs=4) as opool, \
             tc.tile_pool(name="pag", bufs=1, space="PSUM") as pagp, \
             tc.tile_pool(name="pw", bufs=1, space="PSUM") as pwp, \
             tc.tile_pool(name="pt", bufs=1, space="PSUM") as ptp, \
             tc.tile_pool(name="psm", bufs=2, space="PSUM") as psm:

            # ---- constant loads (few big dispatches, split across queues) --
            xnat_sb = cst.tile([128, KT * U], f16, tag="xnat")
            nc.scalar.dma_start(xnat_sb[:], xnat[:, :])
            amat_sb = cst.tile([128, KT * R], f16, tag="amat")
            nc.sync.dma_start(amat_sb[:, :KT * R // 2], amat[:, :KT * R // 2])
            nc.sync.dma_start(amat_sb[:, KT * R // 2:], amat[:, KT * R // 2:])
            xt_sb = cst.tile([128, R], f16, tag="xt")
            nc.scalar.dma_start(xt_sb[:], xt[:, :])
            wts_sb = cst.tile([128, 8 * U], f16, tag="wts")
            nc.scalar.dma_start(wts_sb[:], wts[:, :])
            bias_sb = cst.tile([128, 4], f32, tag="bias")
            nc.scalar.dma_start(bias_sb[:], biases[:, :])
            w12_sb = cst.tile([128, 2], f16, tag="w12")
            nc.scalar.dma_start(w12_sb[:], w12[:, :])
            wfc_sb = cst.tile([128, 1], f16, tag="wfc")
            nc.scalar.dma_start(wfc_sb[:], wfc[:, :])
            bfc_sb = cst.tile([1, 1], f32, tag="bfc")
            nc.scalar.dma_start(bfc_sb[:], bfc[:, :])
            iden_sb = cst.tile([128, 128], f16, tag="iden")
            nc.scalar.dma_start(iden_sb[:], iden[:, :])
            one1_sb = cst.tile([1, 1], f32, tag="one1")
            nc.scalar.dma_start(one1_sb[:], ones1[:, :])
            one128_sb = cst.tile([128, 1], f32, tag="one128")
            nc.scalar.dma_start(one128_sb[:], ones128[:, :])

            hnat = [None, None]   # full natural h (layers 1, 2)
            hT = [None, None]     # transposed own-columns h

            def sage_agg(lhs_sb):
                """agg_T[f, own_i] accumulated over 32 k-chunk matmuls."""
                pag = pagp.tile([128, R], f32, tag="pag")
                for k in range(KT):
                    nc.tensor.matmul(pag[:], lhs_sb[:, k * U:(k + 1) * U],
                                     amat_sb[:, k * R:(k + 1) * R],
                                     start=(k == 0), stop=(k == KT - 1))
                aggT = wrk.tile([128, R], f16, tag="aggT")
                nc.scalar.copy(aggT[:], pag[:])
                return aggT

            # ================= layers 1 and 2 =================
            for l in range(2):
                lhs = xnat_sb if l == 0 else hnat[0]
                rhsT = xt_sb if l == 0 else hT[0]
                aggT = sage_agg(lhs)
                ph = pwp.tile([128, R], f32, tag="ph")
                nc.tensor.matmul(ph[:], wts_sb[:, (2 * l) * U:(2 * l + 1) * U],
                                 aggT[:], start=True, stop=False)
                nc.tensor.matmul(ph[:], wts_sb[:, (2 * l + 1) * U:(2 * l + 2) * U],
                                 rhsT[:], start=False, stop=True)
                hT_new = wrk.tile([128, R], f16, tag=f"hT{l}")
                nc.scalar.activation(hT_new[:], ph[:], AF.Tanh,
                                     bias=bias_sb[:, l:l + 1])
                hT[l] = hT_new
                # transpose own columns back to natural layout
                pt = ptp.tile([128, R], f16, tag="pt")
                for c in range(RC):
                    nc.tensor.transpose(pt[:, c * 128:(c + 1) * 128],
                                        hT_new[:, c * 128:(c + 1) * 128],
                                        iden_sb[:])
                hc = wrk.tile([128, R], f16, tag=f"hc{l}")
                nc.scalar.copy(hc[:], pt[:])
                nc.sync.dma_start(
                    ag_in[l].ap().rearrange("c p f -> p c f"),
                    hc[:].rearrange("p (c f) -> p c f", c=RC))
                nc.gpsimd.collective_compute(
                    "AllGather", mybir.AluOpType.bypass,
                    ins=[ag_in[l].ap().opt()], outs=[ag_out[l].ap().opt()],
                    replica_groups=rgroups)
                hn = big.tile([128, KT * U], f16, tag=f"hnat{l}")
                for r in range(NCORES):
                    eng = nc.sync if r % 2 == 0 else nc.scalar
                    eng.dma_start(
                        hn[:, r * RC * U:(r + 1) * RC * U].rearrange(
                            "p (k f) -> p k f", k=RC),
                        ag_out[l][r].rearrange("k p f -> p k f"))
                hnat[l] = hn

            # ================= heads (shared aggregation) =================
            aggT3 = sage_agg(hnat[1])
            headT = []
            for hi, (wl, wr, bcol) in enumerate(((4, 5, 2), (6, 7, 3))):
                ph = pwp.tile([128, R], f32, tag="ph")
                nc.tensor.matmul(ph[:], wts_sb[:, wl * U:(wl + 1) * U],
                                 aggT3[:], start=True, stop=False)
                nc.tensor.matmul(ph[:], wts_sb[:, wr * U:(wr + 1) * U],
                                 hT[1][:], start=False, stop=True)
                xh = wrk.tile([128, R], f16, tag=f"headT{hi}")
                nc.vector.tensor_scalar_add(xh[:], ph[:], bias_sb[:, bcol:bcol + 1])
                headT.append(xh)

            pab = psm.tile([2, R], f32, tag="small")
            nc.tensor.matmul(pab[:], w12_sb[:], headT[0][:], start=True, stop=True)
            pcp = psm.tile([1, R], f32, tag="small")
            nc.tensor.matmul(pcp[:], wfc_sb[:], headT[1][:], start=True, stop=True)
            abc_own = wrk.tile([2, R], f32, tag="abc_own")
            nc.scalar.copy(abc_own[:], pab[:])
            cp_own = wrk.tile([1, R], f32, tag="cp_own")
            nc.scalar.copy(cp_own[:], pcp[:])
            nc.sync.dma_start(ag3_in[0:2, :], abc_own[:])
            nc.sync.dma_start(ag3_in[2, :], cp_own[:])
            nc.gpsimd.collective_compute(
                "AllGather", mybir.AluOpType.bypass,
                ins=[ag3_in.ap().opt()], outs=[ag3_out.ap().opt()],
                replica_groups=rgroups)

            # ---- global logsumexp of a and b (parallel [128, 32] layout) ----
            abcpm = wrk.tile([128, 3 * 32], f32, tag="abcpm")
            for t in range(3):
                nc.sync.dma_start(abcpm[:, t * 32:(t + 1) * 32],
                                  ag3_out[:, t, :])
            # bb[p, j] = b[j]: broadcast-read DMA straight from ag3_out,
            # quartered so the first output tiles start early; overlaps the
            # LSE chain and never touches GpSimd (no post-collective drain).
            CQ = N // 4
            bb = big.tile([128, N], f32, tag="bb")
            for r in range(NCORES):
                eng = nc.sync if r % 2 == 0 else nc.scalar
                eng.dma_start(bb[:, r * R:(r + 1) * R],
                              ag3_out[r, 1, :].partition_broadcast(128))

            # logsumexp pieces, ACT ops grouped by activation table
            negms, ess, emps = [], [], []
            for t in range(2):
                v = abcpm[:, t * 32:(t + 1) * 32]
                negm = wrk.tile([128, 1], f32, tag=f"negm{t}")
                nc.vector.reduce_max(negm[:], v, axis=AX, negate=True)
                negms.append(negm)
            for t in range(2):
                v = abcpm[:, t * 32:(t + 1) * 32]
                e = wrk.tile([128, 32], f32, tag=f"e{t}")
                es = wrk.tile([128, 1], f32, tag=f"es{t}")
                nc.scalar.activation(e[:], v, AF.Exp, bias=negms[t][:, 0:1],
                                     accum_out=es[:, 0:1])
                ess.append(es)
            for t in range(2):
                emp = wrk.tile([128, 1], f32, tag=f"emp{t}")
                nc.scalar.activation(emp[:], negms[t][:], AF.Exp, scale=-1.0)
                emps.append(emp)
            Ls = []
            for t in range(2):
                # total = sum_p es_p * exp(m_p) as a PE dot product
                ptot = psm.tile([1, 1], f32, tag="small")
                nc.tensor.matmul(ptot[:], ess[t][:], emps[t][:],
                                 start=True, stop=True)
                L = wrk.tile([1, 1], f32, tag=f"L{t}")
                nc.scalar.activation(L[:], ptot[:], AF.Ln)
                Ls.append(L)

            negL2 = wrk.tile([1, 1], f32, tag="negL2")
            nc.vector.tensor_tensor(negL2[:], Ls[0][:], Ls[1][:],
                                    op=mybir.AluOpType.add)
            nc.scalar.mul(negL2[:], negL2[:], -1.0)

            # alpha[i] = a_own[i] - La - Lb, moved to partition axis
            arow = wrk.tile([1, R], f32, tag="arow")
            nc.vector.tensor_scalar_add(arow[:], abc_own[0:1, :],
                                        negL2[0:1, 0:1])
            pa = psm.tile([128, RC], f32, tag="small")
            for c in range(RC):
                nc.tensor.matmul(pa[:, c:c + 1],
                                 arow[0:1, c * 128:(c + 1) * 128],
                                 one1_sb[:], start=True, stop=True)
            alpha = wrk.tile([128, RC], f32, tag="alpha")
            nc.scalar.copy(alpha[:], pa[:])

            # ---- the big output: out[i, j] = b[j] + alpha[i] ----
            di = 0
            for q in range(4):
                for c in range(RC):
                    ob = opool.tile([128, CQ], f32, tag="ob")
                    nc.vector.tensor_scalar_add(ob[:], bb[:, q * CQ:(q + 1) * CQ],
                                                alpha[:, c:c + 1])
                    eng = nc.sync if di % 2 == 0 else nc.scalar
                    di += 1
                    eng.dma_start(
                        out[c * 128:(c + 1) * 128, q * CQ:(q + 1) * CQ],
                        ob[:])

            # critic: tanh(sum(cp)/N + bfc) — off the critical path
            cps = wrk.tile([128, 1], f32, tag="cps")
            nc.vector.reduce_sum(cps[:], abcpm[:, 64:96], axis=AX)
            pct = psm.tile([1, 1], f32, tag="small")
            nc.tensor.matmul(pct[:], one128_sb[:], cps[:], start=True, stop=True)
            crit_sb = wrk.tile([1, 1], f32, tag="crit_sb")
            nc.scalar.activation(crit_sb[:], pct[:], AF.Tanh,
                                 scale=1.0 / N, bias=bfc_sb[0:1, 0:1])
            nc.sync.dma_start(crit[:, :], crit_sb[:])

    nc.compile()
    return nc


def _get_nc():
    if "nc" not in _STATE:
        import concourse.bass as bass  # noqa: F401
        _STATE["nc"] = _build_nc()
    return _STATE["nc"]


def _host_prep(inputs):
    x = np.asarray(inputs["x"], np.float32)
    ei = np.asarray(inputs["edge_index"])
    src = ei[0].astype(np.int64)
    dst = ei[1].astype(np.int64)

    AT = np.zeros((N, N), np.float32)
    np.add.at(AT, (src, dst), 1.0)
    deg = np.bincount(dst, minlength=N).astype(np.float32)
    ATn = AT / np.maximum(deg, 1.0)[None, :]

    wts = np.concatenate([
        inputs["Wf_l"].T, inputs["Wf_r"].T,
        inputs["Wcm_l"].T, inputs["Wcm_r"].T,
        inputs["Wa_l"].T, inputs["Wa_r"].T,
        inputs["Wcr_l"].T, inputs["Wcr_r"].T,
    ], axis=1).astype(fp16)
    biases = np.stack([
        inputs["bf_l"], inputs["bcm_l"], inputs["ba_l"], inputs["bcr_l"],
    ], axis=1).astype(np.float32)
    Wfa = np.asarray(inputs["Wfa"], np.float32)
    w12 = np.stack([Wfa[0, :U], Wfa[0, U:]], axis=1).astype(fp16)
    wfc = np.asarray(inputs["Wfc"], np.float32)[0][:, None].astype(fp16)
    bfc = np.asarray(inputs["bfc"], np.float32).reshape(1, 1)

    common = {
        # partition-major: sbuf[p, k*U+f] = x[k*128+p, f]
        "xnat": np.ascontiguousarray(
            x.astype(fp16).reshape(KT, 128, U).transpose(1, 0, 2).reshape(
                128, KT * U)),
        "wts": wts,
        "biases": biases,
        "w12": w12,
        "wfc": wfc,
        "bfc": bfc,
        "iden": np.eye(128, dtype=fp16),
        "ones1": np.ones((1, 1), np.float32),
        "ones128": np.ones((128, 1), np.float32),
    }
    in_maps = []
    for c in range(NCORES):
        sl = slice(c * R, (c + 1) * R)
        m = dict(common)
        # partition-major: sbuf[p, k*R+i] = ATn[k*128+p, own_i]
        m["amat"] = np.ascontiguousarray(
            ATn[:, sl].astype(fp16).reshape(KT, 128, R).transpose(
                1, 0, 2).reshape(128, KT * R))
        m["xt"] = np.ascontiguousarray(x[sl].T.astype(fp16))
        in_maps.append(m)
    return in_maps


def _run(inputs, trace=False):
    from concourse.bass_utils import run_bass_kernel_spmd
    nc = _get_nc()
    in_maps = _host_prep(inputs)
    res = run_bass_kernel_spmd(nc, in_maps, core_ids=list(range(NCORES)),
                               trace=trace)
    edge_actor = np.concatenate(
        [np.asarray(res.results[c]["out"], np.float32) for c in range(NCORES)],
        axis=0).reshape(N * N, 1)
    edge_critic = np.asarray(res.results[0]["crit"], np.float32).reshape(1, 1)
    return (edge_actor, edge_critic), res


def kernel(**inputs):
    outputs, _ = _run(inputs, trace=False)
    return outputs


# revision 10
# speedup vs baseline: 1.0718x; 1.0256x over previous
"""Distributed Trainium2 Bass kernel for the A2C GNN message-passing model.

Strategy (8 NeuronCores, node-row sharding, 512 rows/core):
  - SAGE aggregation as dense-adjacency matmuls on TensorE: the host builds
    AT[j,i] = count(src=j -> dst=i) / max(indeg(i),1) once; each core keeps its
    512-column slice in SBUF (fp16) and computes agg_T[f, own_i] = sum_k
    h_nat[k-chunk] @ AT[k-chunk] (32 accumulating matmuls, f32 PSUM).
  - Feature maps are kept transposed ([feat, node]) so SAGE biases are
    per-partition activation biases; tanh fused into the PSUM->SBUF copy.
  - After layers 1 and 2 the 512 new rows are AllGathered (fp16, 128KB/rank)
    to rebuild the full natural-layout h for the next aggregation.
  - Actor and critic heads share the layer-3 aggregation. Only the projected
    scalars a = Xa@w1, b = Xa@w2, cp = Xc@wfc are AllGathered (f32, 6KB).
  - The N^2 log_softmax factorizes: out[i,j] = a_i + b_j + bfa - LSE with
    LSE = bfa + logsumexp(a) + logsumexp(b), so bfa cancels and
    out[i,j] = b_j + (a_i - La - Lb). Each core writes its 512x4096 f32 block:
    partition-broadcast of the b row + one tensor_scalar add per tile.
  - edge_critic = tanh(mean(cp) + bfc) (the (cp_i+cp_j)/2 mean collapses).

Perf notes (trace-driven):
  - A dummy AllGather with no dependencies runs first so the model-entry CC
    barrier / stream spin-up / cold RDH overlap the HBM load phase instead of
    stalling the first real collective.
  - amat/xnat are staged in DRAM partition-major so each loads as ONE
    dma_start (dispatch on the Sync/Act queues costs ~0.4-2.4us per DMA).
  - The post-AllGather h reload is split per rank-block so layer k+1's
    accumulation matmuls start while later blocks are still in flight.
  - The output is built in 16 [128,1024] tiles: b-row broadcast quarters on
    GpSimd pipeline with DVE adds and output DMAs alternating Sync/Act.
"""

import numpy as np

N = 4096
U = 128
NCORES = 8
R = N // NCORES          # 512 rows per core
KT = N // 128            # 32 contraction chunks
RC = R // 128            # 4 row chunks per core

fp16 = np.float16

_STATE = {}


def _build_nc():
    import concourse.bass as bass
    import concourse.bacc as bacc
    import concourse.mybir as mybir
    import concourse.tile as tile

    f32 = mybir.dt.float32
    f16 = mybir.dt.float16
    AX = mybir.AxisListType.X
    AF = mybir.ActivationFunctionType

    nc = bacc.Bacc("TRN2", target_bir_lowering=False, debug=False,
                   num_devices=NCORES)

    # ---- kernel I/O ----
    amat = nc.dram_tensor("amat", [128, KT * R], f16, kind="ExternalInput")
    xnat = nc.dram_tensor("xnat", [128, KT * U], f16, kind="ExternalInput")
    xt = nc.dram_tensor("xt", [U, R], f16, kind="ExternalInput")
    wts = nc.dram_tensor("wts", [U, 8 * U], f16, kind="ExternalInput")
    biases = nc.dram_tensor("biases", [U, 4], f32, kind="ExternalInput")
    w12 = nc.dram_tensor("w12", [U, 2], f16, kind="ExternalInput")
    wfc = nc.dram_tensor("wfc", [U, 1], f16, kind="ExternalInput")
    bfc = nc.dram_tensor("bfc", [1, 1], f32, kind="ExternalInput")
    iden = nc.dram_tensor("iden", [128, 128], f16, kind="ExternalInput")
    ones1 = nc.dram_tensor("ones1", [1, 1], f32, kind="ExternalInput")
    ones128 = nc.dram_tensor("ones128", [128, 1], f32, kind="ExternalInput")

    out = nc.dram_tensor("out", [R, N], f32, kind="ExternalOutput")
    crit = nc.dram_tensor("crit", [1, 1], f32, kind="ExternalOutput")

    # ---- collective bounce buffers (internal DRAM) ----
    ag_in = [nc.dram_tensor(f"ag{l}_in", [RC, 128, U], f16) for l in (1, 2)]
    ag_out = [nc.dram_tensor(f"ag{l}_out", [NCORES, RC, 128, U], f16,
                             addr_space="Shared") for l in (1, 2)]
    ag3_in = nc.dram_tensor("ag3_in", [3, R], f32)
    ag3_out = nc.dram_tensor("ag3_out", [NCORES, 3, R], f32,
                             addr_space="Shared")
    rgroups = [list(range(NCORES))]

    with tile.TileContext(nc) as tc:
        with tc.tile_pool(name="const", bufs=1) as cst, \
             tc.tile_pool(name="work", bufs=1) as wrk, \
             tc.tile_pool(name="big", bufs=1) as big, \
             tc.tile_pool(name="opool", bufs=4) as opool, \
             tc.tile_pool(name="pag", bufs=1, space="PSUM") as pagp, \
             tc.tile_pool(name="pw", bufs=1, space="PSUM") as pwp, \
             tc.tile_pool(name="pt", bufs=1, space="PSUM") as ptp, \
             tc.tile_pool(name="psm", bufs=2, space="PSUM") as psm:

            # ---- constant loads (few big dispatches, split across queues) --
            xnat_sb = cst.tile([128, KT * U], f16, tag="xnat")
            nc.scalar.dma_start(xnat_sb[:], xnat[:, :])
            amat_sb = cst.tile([128, KT * R], f16, tag="amat")
            nc.sync.dma_start(amat_sb[:, :KT * R // 2], amat[:, :KT * R // 2])
            nc.sync.dma_start(amat_sb[:, KT * R // 2:], amat[:, KT * R // 2:])
            xt_sb = cst.tile([128, R], f16, tag="xt")
            nc.scalar.dma_start(xt_sb[:], xt[:, :])
            wts_sb = cst.tile([128, 8 * U], f16, tag="wts")
            nc.scalar.dma_start(wts_sb[:], wts[:, :])
            bias_sb = cst.tile([128, 4], f32, tag="bias")
            nc.scalar.dma_start(bias_sb[:], biases[:, :])
            w12_sb = cst.tile([128, 2], f16, tag="w12")
            nc.scalar.dma_start(w12_sb[:], w12[:, :])
            wfc_sb = cst.tile([128, 1], f16, tag="wfc")
            nc.scalar.dma_start(wfc_sb[:], wfc[:, :])
            bfc_sb = cst.tile([1, 1], f32, tag="bfc")
            nc.scalar.dma_start(bfc_sb[:], bfc[:, :])
            iden_sb = cst.tile([128, 128], f16, tag="iden")
            nc.scalar.dma_start(iden_sb[:], iden[:, :])
            one1_sb = cst.tile([1, 1], f32, tag="one1")
            nc.scalar.dma_start(one1_sb[:], ones1[:, :])
            one128_sb = cst.tile([128, 1], f32, tag="one128")
            nc.scalar.dma_start(one128_sb[:], ones128[:, :])

            hnat = [None, None]   # full natural h (layers 1, 2)
            hT = [None, None]     # transposed own-columns h

            def sage_agg(lhs_sb):
                """agg_T[f, own_i] accumulated over 32 k-chunk matmuls."""
                pag = pagp.tile([128, R], f32, tag="pag")
                for k in range(KT):
                    nc.tensor.matmul(pag[:], lhs_sb[:, k * U:(k + 1) * U],
                                     amat_sb[:, k * R:(k + 1) * R],
                                     start=(k == 0), stop=(k == KT - 1))
                aggT = wrk.tile([128, R], f16, tag="aggT")
                nc.scalar.copy(aggT[:], pag[:])
                return aggT

            # ================= layers 1 and 2 =================
            for l in range(2):
                lhs = xnat_sb if l == 0 else hnat[0]
                rhsT = xt_sb if l == 0 else hT[0]
                aggT = sage_agg(lhs)
                ph = pwp.tile([128, R], f32, tag="ph")
                nc.tensor.matmul(ph[:], wts_sb[:, (2 * l) * U:(2 * l + 1) * U],
                                 aggT[:], start=True, stop=False)
                nc.tensor.matmul(ph[:], wts_sb[:, (2 * l + 1) * U:(2 * l + 2) * U],
                                 rhsT[:], start=False, stop=True)
                hT_new = wrk.tile([128, R], f16, tag=f"hT{l}")
                nc.scalar.activation(hT_new[:], ph[:], AF.Tanh,
                                     bias=bias_sb[:, l:l + 1])
                hT[l] = hT_new
                # transpose own columns back to natural layout
                pt = ptp.tile([128, R], f16, tag="pt")
                for c in range(RC):
                    nc.tensor.transpose(pt[:, c * 128:(c + 1) * 128],
                                        hT_new[:, c * 128:(c + 1) * 128],
                                        iden_sb[:])
                hc = wrk.tile([128, R], f16, tag=f"hc{l}")
                nc.scalar.copy(hc[:], pt[:])
                nc.sync.dma_start(
                    ag_in[l].ap().rearrange("c p f -> p c f"),
                    hc[:].rearrange("p (c f) -> p c f", c=RC))
                nc.gpsimd.collective_compute(
                    "AllGather", mybir.AluOpType.bypass,
                    ins=[ag_in[l].ap().opt()], outs=[ag_out[l].ap().opt()],
                    replica_groups=rgroups)
                hn = big.tile([128, KT * U], f16, tag=f"hnat{l}")
                for r in range(NCORES):
                    for h, eng in enumerate((nc.sync, nc.scalar)):
                        o = r * RC * U + h * (RC // 2) * U
                        eng.dma_start(
                            hn[:, o:o + (RC // 2) * U].rearrange(
                                "p (k f) -> p k f", k=RC // 2),
                            ag_out[l][r][h * (RC // 2):(h + 1) * (RC // 2)]
                            .rearrange("k p f -> p k f"))
                hnat[l] = hn

            # ================= heads (shared aggregation) =================
            aggT3 = sage_agg(hnat[1])
            headT = []
            for hi, (wl, wr, bcol) in enumerate(((4, 5, 2), (6, 7, 3))):
                ph = pwp.tile([128, R], f32, tag="ph")
                nc.tensor.matmul(ph[:], wts_sb[:, wl * U:(wl + 1) * U],
                                 aggT3[:], start=True, stop=False)
                nc.tensor.matmul(ph[:], wts_sb[:, wr * U:(wr + 1) * U],
                                 hT[1][:], start=False, stop=True)
                xh = wrk.tile([128, R], f16, tag=f"headT{hi}")
                nc.vector.tensor_scalar_add(xh[:], ph[:], bias_sb[:, bcol:bcol + 1])
                headT.append(xh)

            pab = psm.tile([2, R], f32, tag="small")
            nc.tensor.matmul(pab[:], w12_sb[:], headT[0][:], start=True, stop=True)
            pcp = psm.tile([1, R], f32, tag="small")
            nc.tensor.matmul(pcp[:], wfc_sb[:], headT[1][:], start=True, stop=True)
            abc_own = wrk.tile([2, R], f32, tag="abc_own")
            nc.vector.tensor_copy(abc_own[:], pab[:])
            cp_own = wrk.tile([1, R], f32, tag="cp_own")
            nc.vector.tensor_copy(cp_own[:], pcp[:])
            # dummy Exp pre-loads the ACT exp/ln table before the AllGather
            # so the post-gather logsumexp chain runs table-warm
            dum = wrk.tile([1, 1], f32, tag="dum")
            nc.scalar.activation(dum[:], one1_sb[:], AF.Exp)
            nc.sync.dma_start(ag3_in[0:2, :], abc_own[:])
            nc.sync.dma_start(ag3_in[2, :], cp_own[:])
            nc.gpsimd.collective_compute(
                "AllGather", mybir.AluOpType.bypass,
                ins=[ag3_in.ap().opt()], outs=[ag3_out.ap().opt()],
                replica_groups=rgroups)

            # ---- global logsumexp of a and b (parallel [128, 32] layout) ----
            abcpm = wrk.tile([128, 3 * 32], f32, tag="abcpm")
            for t in range(3):
                nc.sync.dma_start(abcpm[:, t * 32:(t + 1) * 32],
                                  ag3_out[:, t, :])
            # bb[p, j] = b[j]: broadcast-read DMA straight from ag3_out,
            # quartered so the first output tiles start early; overlaps the
            # LSE chain and never touches GpSimd (no post-collective drain).
            CQ = N // 4
            bb = big.tile([128, N], f32, tag="bb")
            for r in range(NCORES):
                eng = nc.sync if r % 2 == 0 else nc.scalar
                eng.dma_start(bb[:, r * R:(r + 1) * R],
                              ag3_out[r, 1, :].partition_broadcast(128))

            # logsumexp pieces, ACT ops grouped by activation table
            negms, ess, emps = [], [], []
            for t in range(2):
                v = abcpm[:, t * 32:(t + 1) * 32]
                negm = wrk.tile([128, 1], f32, tag=f"negm{t}")
                nc.vector.reduce_max(negm[:], v, axis=AX, negate=True)
                negms.append(negm)
            for t in range(2):
                v = abcpm[:, t * 32:(t + 1) * 32]
                e = wrk.tile([128, 32], f32, tag=f"e{t}")
                es = wrk.tile([128, 1], f32, tag=f"es{t}")
                nc.scalar.activation(e[:], v, AF.Exp, bias=negms[t][:, 0:1],
                                     accum_out=es[:, 0:1])
                ess.append(es)
            for t in range(2):
                emp = wrk.tile([128, 1], f32, tag=f"emp{t}")
                nc.scalar.activation(emp[:], negms[t][:], AF.Exp, scale=-1.0)
                emps.append(emp)
            Ls = []
            for t in range(2):
                # total = sum_p es_p * exp(m_p) as a PE dot product
                ptot = psm.tile([1, 1], f32, tag="small")
                nc.tensor.matmul(ptot[:], ess[t][:], emps[t][:],
                                 start=True, stop=True)
                L = wrk.tile([1, 1], f32, tag=f"L{t}")
                nc.scalar.activation(L[:], ptot[:], AF.Ln)
                Ls.append(L)

            negL2 = wrk.tile([1, 1], f32, tag="negL2")
            nc.vector.tensor_tensor(negL2[:], Ls[0][:], Ls[1][:],
                                    op=mybir.AluOpType.add)
            nc.scalar.mul(negL2[:], negL2[:], -1.0)

            # alpha[i] = a_own[i] - La - Lb, moved to partition axis
            arow = wrk.tile([1, R], f32, tag="arow")
            nc.vector.tensor_scalar_add(arow[:], abc_own[0:1, :],
                                        negL2[0:1, 0:1])
            pa = psm.tile([128, RC], f32, tag="small")
            for c in range(RC):
                nc.tensor.matmul(pa[:, c:c + 1],
                                 arow[0:1, c * 128:(c + 1) * 128],
                                 one1_sb[:], start=True, stop=True)
            alpha = wrk.tile([128, RC], f32, tag="alpha")
            nc.vector.tensor_copy(alpha[:], pa[:])

            # ---- the big output: out[i, j] = b[j] + alpha[i] ----
            di = 0
            for q in range(4):
                for c in range(RC):
                    ob = opool.tile([128, CQ], f32, tag="ob")
                    nc.vector.tensor_scalar_add(ob[:], bb[:, q * CQ:(q + 1) * CQ],
                                                alpha[:, c:c + 1])
                    di += 1
                    half = CQ // 2
                    for h, eng in enumerate((nc.sync, nc.scalar)):
                        eng.dma_start(
                            out[c * 128:(c + 1) * 128,
                                q * CQ + h * half:q * CQ + (h + 1) * half],
                            ob[:, h * half:(h + 1) * half])

            # critic: tanh(sum(cp)/N + bfc) — off the critical path
            cps = wrk.tile([128, 1], f32, tag="cps")
            nc.vector.reduce_sum(cps[:], abcpm[:, 64:96], axis=AX)
            pct = psm.tile([1, 1], f32, tag="small")
            nc.tensor.matmul(pct[:], one128_sb[:], cps[:], start=True, stop=True)
            crit_sb = wrk.tile([1, 1], f32, tag="crit_sb")
            nc.scalar.activation(crit_sb[:], pct[:], AF.Tanh,
                                 scale=1.0 / N, bias=bfc_sb[0:1, 0:1])
            nc.sync.dma_start(crit[:, :], crit_sb[:])

    nc.compile()
    return nc


def _get_nc():
    if "nc" not in _STATE:
        import concourse.bass as bass  # noqa: F401
        _STATE["nc"] = _build_nc()
    return _STATE["nc"]


def _host_prep(inputs):
    x = np.asarray(inputs["x"], np.float32)
    ei = np.asarray(inputs["edge_index"])
    src = ei[0].astype(np.int64)
    dst = ei[1].astype(np.int64)

    AT = np.zeros((N, N), np.float32)
    np.add.at(AT, (src, dst), 1.0)
    deg = np.bincount(dst, minlength=N).astype(np.float32)
    ATn = AT / np.maximum(deg, 1.0)[None, :]

    wts = np.concatenate([
        inputs["Wf_l"].T, inputs["Wf_r"].T,
        inputs["Wcm_l"].T, inputs["Wcm_r"].T,
        inputs["Wa_l"].T, inputs["Wa_r"].T,
        inputs["Wcr_l"].T, inputs["Wcr_r"].T,
    ], axis=1).astype(fp16)
    biases = np.stack([
        inputs["bf_l"], inputs["bcm_l"], inputs["ba_l"], inputs["bcr_l"],
    ], axis=1).astype(np.float32)
    Wfa = np.asarray(inputs["Wfa"], np.float32)
    w12 = np.stack([Wfa[0, :U], Wfa[0, U:]], axis=1).astype(fp16)
    wfc = np.asarray(inputs["Wfc"], np.float32)[0][:, None].astype(fp16)
    bfc = np.asarray(inputs["bfc"], np.float32).reshape(1, 1)

    common = {
        # partition-major: sbuf[p, k*U+f] = x[k*128+p, f]
        "xnat": np.ascontiguousarray(
            x.astype(fp16).reshape(KT, 128, U).transpose(1, 0, 2).reshape(
                128, KT * U)),
        "wts": wts,
        "biases": biases,
        "w12": w12,
        "wfc": wfc,
        "bfc": bfc,
        "iden": np.eye(128, dtype=fp16),
        "ones1": np.ones((1, 1), np.float32),
        "ones128": np.ones((128, 1), np.float32),
    }
    in_maps = []
    for c in range(NCORES):
        sl = slice(c * R, (c + 1) * R)
        m = dict(common)
        # partition-major: sbuf[p, k*R+i] = ATn[k*128+p, own_i]
        m["amat"] = np.ascontiguousarray(
            ATn[:, sl].astype(fp16).reshape(KT, 128, R).transpose(
                1, 0, 2).reshape(128, KT * R))
        m["xt"] = np.ascontiguousarray(x[sl].T.astype(fp16))
        in_maps.append(m)
    return in_maps


def _run(inputs, trace=False):
    from concourse.bass_utils import run_bass_kernel_spmd
    nc = _get_nc()
    in_maps = _host_prep(inputs)
    res = run_bass_kernel_spmd(nc, in_maps, core_ids=list(range(NCORES)),
                               trace=trace)
    edge_actor = np.concatenate(
        [np.asarray(res.results[c]["out"], np.float32) for c in range(NCORES)],
        axis=0).reshape(N * N, 1)
    edge_critic = np.asarray(res.results[0]["crit"], np.float32).reshape(1, 1)
    return (edge_actor, edge_critic), res


def kernel(**inputs):
    outputs, _ = _run(inputs, trace=False)
    return outputs


# revision 13
# speedup vs baseline: 1.1555x; 1.0781x over previous
"""Distributed Trainium2 Bass kernel for the A2C GNN message-passing model.

Strategy (8 NeuronCores, node-row sharding, 512 rows/core):
  - SAGE aggregation as dense-adjacency matmuls on TensorE: the host builds
    AT[j,i] = count(src=j -> dst=i) / max(indeg(i),1) once; each core keeps its
    512-column slice in SBUF (fp16) and computes agg_T[f, own_i] = sum_k
    h_nat[k-chunk] @ AT[k-chunk] (32 accumulating matmuls, f32 PSUM).
  - Feature maps are kept transposed ([feat, node]) so SAGE biases are
    per-partition activation biases; tanh fused into the PSUM->SBUF copy.
  - After layers 1 and 2 the 512 new rows are AllGathered (fp16, 128KB/rank)
    to rebuild the full natural-layout h for the next aggregation.
  - Actor and critic heads share the layer-3 aggregation. Only the projected
    scalars a = Xa@w1, b = Xa@w2, cp = Xc@wfc are AllGathered (f32, 6KB).
  - The N^2 log_softmax factorizes: out[i,j] = a_i + b_j + bfa - LSE with
    LSE = bfa + logsumexp(a) + logsumexp(b), so bfa cancels and
    out[i,j] = b_j + (a_i - La - Lb). Each core writes its 512x4096 f32 block:
    partition-broadcast of the b row + one tensor_scalar add per tile.
  - edge_critic = tanh(mean(cp) + bfc) (the (cp_i+cp_j)/2 mean collapses).

Perf notes (trace-driven):
  - A dummy AllGather with no dependencies runs first so the model-entry CC
    barrier / stream spin-up / cold RDH overlap the HBM load phase instead of
    stalling the first real collective.
  - amat/xnat are staged in DRAM partition-major so each loads as ONE
    dma_start (dispatch on the Sync/Act queues costs ~0.4-2.4us per DMA).
  - The post-AllGather h reload is split per rank-block so layer k+1's
    accumulation matmuls start while later blocks are still in flight.
  - The output is built in 16 [128,1024] tiles: b-row broadcast quarters on
    GpSimd pipeline with DVE adds and output DMAs alternating Sync/Act.
"""

import numpy as np

N = 4096
U = 128
NCORES = 8
R = N // NCORES          # 512 rows per core
KT = N // 128            # 32 contraction chunks
RC = R // 128            # 4 row chunks per core

fp16 = np.float16

_STATE = {}


def _build_nc():
    import concourse.bass as bass
    import concourse.bacc as bacc
    import concourse.mybir as mybir
    import concourse.tile as tile

    f32 = mybir.dt.float32
    f16 = mybir.dt.float16
    AX = mybir.AxisListType.X
    AF = mybir.ActivationFunctionType

    nc = bacc.Bacc("TRN2", target_bir_lowering=False, debug=False,
                   num_devices=NCORES)

    # ---- kernel I/O ----
    amat = nc.dram_tensor("amat", [128, KT * R], f16, kind="ExternalInput")
    xnat = nc.dram_tensor("xnat", [128, KT * U], f16, kind="ExternalInput")
    xt = nc.dram_tensor("xt", [U, R], f16, kind="ExternalInput")
    wts = nc.dram_tensor("wts", [U, 4 * U], f16, kind="ExternalInput")
    biases = nc.dram_tensor("biases", [U, 2], f32, kind="ExternalInput")
    u3 = nc.dram_tensor("u3", [U, 3], f16, kind="ExternalInput")
    v3 = nc.dram_tensor("v3", [U, 3], f16, kind="ExternalInput")
    c3 = nc.dram_tensor("c3", [3, 1], f32, kind="ExternalInput")
    bfc = nc.dram_tensor("bfc", [1, 1], f32, kind="ExternalInput")
    iden = nc.dram_tensor("iden", [128, 128], f16, kind="ExternalInput")
    ones1 = nc.dram_tensor("ones1", [1, 1], f32, kind="ExternalInput")
    ones128 = nc.dram_tensor("ones128", [128, 1], f32, kind="ExternalInput")

    out = nc.dram_tensor("out", [R, N], f32, kind="ExternalOutput")
    crit = nc.dram_tensor("crit", [1, 1], f32, kind="ExternalOutput")

    # ---- collective bounce buffers (internal DRAM) ----
    ag_in = [nc.dram_tensor("ag1_in", [RC, 128, U], f16)]
    ag_out = [nc.dram_tensor("ag1_out", [NCORES, RC, 128, U], f16,
                             addr_space="Shared")]
    ags_in = nc.dram_tensor("ags_in", [128, 12], f16)
    ags_out = nc.dram_tensor("ags_out", [NCORES, 128, 12], f16,
                             addr_space="Shared")
    ag3_in = nc.dram_tensor("ag3_in", [3, R], f32)
    ag3_out = nc.dram_tensor("ag3_out", [NCORES, 3, R], f32,
                             addr_space="Shared")
    rgroups = [list(range(NCORES))]

    with tile.TileContext(nc) as tc:
        with tc.tile_pool(name="const", bufs=1) as cst, \
             tc.tile_pool(name="work", bufs=1) as wrk, \
             tc.tile_pool(name="big", bufs=1) as big, \
             tc.tile_pool(name="opool", bufs=4) as opool, \
             tc.tile_pool(name="pag", bufs=1, space="PSUM") as pagp, \
             tc.tile_pool(name="pw", bufs=1, space="PSUM") as pwp, \
             tc.tile_pool(name="pt", bufs=1, space="PSUM") as ptp, \
             tc.tile_pool(name="psm", bufs=1, space="PSUM") as psm:

            # ---- constant loads (few big dispatches, split across queues) --
            xnat_sb = cst.tile([128, KT * U], f16, tag="xnat")
            nc.scalar.dma_start(xnat_sb[:], xnat[:, :])
            amat_sb = cst.tile([128, KT * R], f16, tag="amat")
            nc.sync.dma_start(amat_sb[:, :KT * R // 2], amat[:, :KT * R // 2])
            nc.sync.dma_start(amat_sb[:, KT * R // 2:], amat[:, KT * R // 2:])
            xt_sb = cst.tile([128, R], f16, tag="xt")
            nc.scalar.dma_start(xt_sb[:], xt[:, :])
            wts_sb = cst.tile([128, 4 * U], f16, tag="wts")
            nc.scalar.dma_start(wts_sb[:], wts[:, :])
            bias_sb = cst.tile([128, 2], f32, tag="bias")
            nc.scalar.dma_start(bias_sb[:], biases[:, :])
            u3_sb = cst.tile([128, 3], f16, tag="u3")
            nc.scalar.dma_start(u3_sb[:], u3[:, :])
            v3_sb = cst.tile([128, 3], f16, tag="v3")
            nc.scalar.dma_start(v3_sb[:], v3[:, :])
            c3_sb = cst.tile([3, 1], f32, tag="c3")
            nc.scalar.dma_start(c3_sb[:], c3[:, :])
            bfc_sb = cst.tile([1, 1], f32, tag="bfc")
            nc.scalar.dma_start(bfc_sb[:], bfc[:, :])
            iden_sb = cst.tile([128, 128], f16, tag="iden")
            nc.scalar.dma_start(iden_sb[:], iden[:, :])
            one1_sb = cst.tile([1, 1], f32, tag="one1")
            nc.scalar.dma_start(one1_sb[:], ones1[:, :])
            one128_sb = cst.tile([128, 1], f32, tag="one128")
            nc.scalar.dma_start(one128_sb[:], ones128[:, :])

            hT = [None, None]     # transposed own-columns h

            def sage_agg(lhs_sb):
                """agg_T[f, own_i] accumulated over 32 k-chunk matmuls."""
                pag = pagp.tile([128, R], f32, tag="pag")
                for k in range(KT):
                    nc.tensor.matmul(pag[:], lhs_sb[:, k * U:(k + 1) * U],
                                     amat_sb[:, k * R:(k + 1) * R],
                                     start=(k == 0), stop=(k == KT - 1))
                aggT = wrk.tile([128, R], f16, tag="aggT")
                nc.scalar.copy(aggT[:], pag[:])
                return aggT

            # ================= layers 1 and 2 =================
            hn1 = None
            for l in range(2):
                lhs = xnat_sb if l == 0 else hn1
                rhsT = xt_sb if l == 0 else hT[0]
                aggT = sage_agg(lhs)
                ph = pwp.tile([128, R], f32, tag="ph")
                nc.tensor.matmul(ph[:], wts_sb[:, (2 * l) * U:(2 * l + 1) * U],
                                 aggT[:], start=True, stop=False)
                nc.tensor.matmul(ph[:], wts_sb[:, (2 * l + 1) * U:(2 * l + 2) * U],
                                 rhsT[:], start=False, stop=True)
                hT_new = wrk.tile([128, R], f16, tag=f"hT{l}")
                nc.scalar.activation(hT_new[:], ph[:], AF.Tanh,
                                     bias=bias_sb[:, l:l + 1])
                hT[l] = hT_new
                if l == 0:
                    # transpose own columns back to natural layout and gather
                    pt = ptp.tile([128, R], f16, tag="pt")
                    for c in range(RC):
                        nc.tensor.transpose(pt[:, c * 128:(c + 1) * 128],
                                            hT_new[:, c * 128:(c + 1) * 128],
                                            iden_sb[:])
                    hc = wrk.tile([128, R], f16, tag="hc0")
                    nc.scalar.copy(hc[:], pt[:])
                    nc.sync.dma_start(
                        ag_in[0].ap().rearrange("c p f -> p c f"),
                        hc[:].rearrange("p (c f) -> p c f", c=RC))
                    nc.gpsimd.collective_compute(
                        "AllGather", mybir.AluOpType.bypass,
                        ins=[ag_in[0].ap().opt()], outs=[ag_out[0].ap().opt()],
                        replica_groups=rgroups)
                    hn1 = big.tile([128, KT * U], f16, tag="hnat0")
                    for r in range(NCORES):
                        for h, eng in enumerate((nc.sync, nc.scalar)):
                            o = r * RC * U + h * (RC // 2) * U
                            eng.dma_start(
                                hn1[:, o:o + (RC // 2) * U].rearrange(
                                    "p (k f) -> p k f", k=RC // 2),
                                ag_out[0][r][h * (RC // 2):(h + 1) * (RC // 2)]
                                .rearrange("k p f -> p k f"))

            # ============== scalarized heads ==============
            # a_i = sum_j A^[i,j] s1_j + c1 + v1.h2_i with s_kj = u_k.h2_j;
            # only the 3 s-scalars per node are gathered, not h2 itself.
            hT2 = hT[1]
            pspm = psm.tile([128, 12], f32, tag="pspm")
            for c in range(RC):
                nc.tensor.matmul(pspm[:, c * 3:(c + 1) * 3],
                                 hT2[:, c * 128:(c + 1) * 128], u3_sb[:],
                                 start=True, stop=True)
            pt3 = psm.tile([3, R], f32, tag="pt3")
            nc.tensor.matmul(pt3[:], v3_sb[:], hT2[:], start=True, stop=True)
            t3_sb = wrk.tile([3, R], f32, tag="t3_sb")
            nc.vector.tensor_copy(t3_sb[:], pt3[:])
            s_sb = wrk.tile([128, 12], f16, tag="s_sb")
            nc.vector.tensor_copy(s_sb[:], pspm[:])
            nc.sync.dma_start(ags_in[:, :], s_sb[:])
            # dummy Exp pre-loads the ACT exp/ln table before the gathers so
            # the post-gather logsumexp chain runs table-warm
            dum = wrk.tile([1, 1], f32, tag="dum")
            nc.scalar.activation(dum[:], one1_sb[:], AF.Exp)
            nc.gpsimd.collective_compute(
                "AllGather", mybir.AluOpType.bypass,
                ins=[ags_in.ap().opt()], outs=[ags_out.ap().opt()],
                replica_groups=rgroups)
            s_all = wrk.tile([128, NCORES * 12], f16, tag="s_all")
            nc.scalar.dma_start(s_all[:].rearrange("p (r w) -> p r w", r=NCORES),
                                ags_out.ap().rearrange("r p w -> p r w"))
            pagS = psm.tile([3, R], f32, tag="pagS")
            for k in range(KT):
                nc.tensor.matmul(pagS[:], s_all[:, k * 3:(k + 1) * 3],
                                 amat_sb[:, k * R:(k + 1) * R],
                                 start=(k == 0), stop=(k == KT - 1))
            abc_own = wrk.tile([3, R], f32, tag="abc_own")
            nc.vector.tensor_tensor(abc_own[:], pagS[:], t3_sb[:],
                                    op=mybir.AluOpType.add)
            nc.vector.tensor_scalar_add(abc_own[:], abc_own[:], c3_sb[:, 0:1])
            nc.sync.dma_start(ag3_in[:, :], abc_own[:])
            nc.gpsimd.collective_compute(
                "AllGather", mybir.AluOpType.bypass,
                ins=[ag3_in.ap().opt()], outs=[ag3_out.ap().opt()],
                replica_groups=rgroups)

            # ---- global logsumexp of a and b (parallel [128, 32] layout) ----
            abcpm = wrk.tile([128, 3 * 32], f32, tag="abcpm")
            for t in range(3):
                nc.sync.dma_start(abcpm[:, t * 32:(t + 1) * 32],
                                  ag3_out[:, t, :])
            # bb[p, j] = b[j]: broadcast-read DMA straight from ag3_out,
            # quartered so the first output tiles start early; overlaps the
            # LSE chain and never touches GpSimd (no post-collective drain).
            CQ = N // 4
            bb = big.tile([128, N], f32, tag="bb")
            for r in range(NCORES):
                eng = nc.sync if r % 2 == 0 else nc.scalar
                eng.dma_start(bb[:, r * R:(r + 1) * R],
                              ag3_out[r, 1, :].partition_broadcast(128))

            # logsumexp pieces, ACT ops grouped by activation table
            negms, ess, emps = [], [], []
            for t in range(2):
                v = abcpm[:, t * 32:(t + 1) * 32]
                negm = wrk.tile([128, 1], f32, tag=f"negm{t}")
                nc.vector.reduce_max(negm[:], v, axis=AX, negate=True)
                negms.append(negm)
            for t in range(2):
                v = abcpm[:, t * 32:(t + 1) * 32]
                e = wrk.tile([128, 32], f32, tag=f"e{t}")
                es = wrk.tile([128, 1], f32, tag=f"es{t}")
                nc.scalar.activation(e[:], v, AF.Exp, bias=negms[t][:, 0:1],
                                     accum_out=es[:, 0:1])
                ess.append(es)
            for t in range(2):
                emp = wrk.tile([128, 1], f32, tag=f"emp{t}")
                nc.scalar.activation(emp[:], negms[t][:], AF.Exp, scale=-1.0)
                emps.append(emp)
            Ls = []
            for t in range(2):
                # total = sum_p es_p * exp(m_p) as a PE dot product
                ptot = psm.tile([1, 1], f32, tag="small")
                nc.tensor.matmul(ptot[:], ess[t][:], emps[t][:],
                                 start=True, stop=True)
                L = wrk.tile([1, 1], f32, tag=f"L{t}")
                nc.scalar.activation(L[:], ptot[:], AF.Ln)
                Ls.append(L)

            negL2 = wrk.tile([1, 1], f32, tag="negL2")
            nc.vector.tensor_tensor(negL2[:], Ls[0][:], Ls[1][:],
                                    op=mybir.AluOpType.add)
            nc.scalar.mul(negL2[:], negL2[:], -1.0)

            # alpha[i] = a_own[i] - La - Lb, moved to partition axis
            arow = wrk.tile([1, R], f32, tag="arow")
            nc.vector.tensor_scalar_add(arow[:], abc_own[0:1, :],
                                        negL2[0:1, 0:1])
            pa = psm.tile([128, RC], f32, tag="small")
            for c in range(RC):
                nc.tensor.matmul(pa[:, c:c + 1],
                                 arow[0:1, c * 128:(c + 1) * 128],
                                 one1_sb[:], start=True, stop=True)
            alpha = wrk.tile([128, RC], f32, tag="alpha")
            nc.vector.tensor_copy(alpha[:], pa[:])

            # ---- the big output: out[i, j] = b[j] + alpha[i] ----
            di = 0
            for q in range(4):
                for c in range(RC):
                    ob = opool.tile([128, CQ], f32, tag="ob")
                    nc.vector.tensor_scalar_add(ob[:], bb[:, q * CQ:(q + 1) * CQ],
                                                alpha[:, c:c + 1])
                    di += 1
                    half = CQ // 2
                    for h, eng in enumerate((nc.sync, nc.scalar)):
                        eng.dma_start(
                            out[c * 128:(c + 1) * 128,
                                q * CQ + h * half:q * CQ + (h + 1) * half],
                            ob[:, h * half:(h + 1) * half])

            # critic: tanh(sum(cp)/N + bfc) — off the critical path
            cps = wrk.tile([128, 1], f32, tag="cps")
            nc.vector.reduce_sum(cps[:], abcpm[:, 64:96], axis=AX)
            pct = psm.tile([1, 1], f32, tag="small")
            nc.tensor.matmul(pct[:], one128_sb[:], cps[:], start=True, stop=True)
            crit_sb = wrk.tile([1, 1], f32, tag="crit_sb")
            nc.scalar.activation(crit_sb[:], pct[:], AF.Tanh,
                                 scale=1.0 / N, bias=bfc_sb[0:1, 0:1])
            nc.sync.dma_start(crit[:, :], crit_sb[:])

    nc.compile()
    return nc


def _get_nc():
    if "nc" not in _STATE:
        import concourse.bass as bass  # noqa: F401
        _STATE["nc"] = _build_nc()
    return _STATE["nc"]


def _host_prep(inputs):
    x = np.asarray(inputs["x"], np.float32)
    ei = np.asarray(inputs["edge_index"])
    src = ei[0].astype(np.int64)
    dst = ei[1].astype(np.int64)

    AT = np.zeros((N, N), np.float32)
    np.add.at(AT, (src, dst), 1.0)
    deg = np.bincount(dst, minlength=N).astype(np.float32)
    ATn = AT / np.maximum(deg, 1.0)[None, :]

    wts = np.concatenate([
        inputs["Wf_l"].T, inputs["Wf_r"].T,
        inputs["Wcm_l"].T, inputs["Wcm_r"].T,
    ], axis=1).astype(fp16)
    biases = np.stack([
        inputs["bf_l"], inputs["bcm_l"],
    ], axis=1).astype(np.float32)
    Wfa = np.asarray(inputs["Wfa"], np.float32)
    w1, w2 = Wfa[0, :U], Wfa[0, U:]
    wfc_row = np.asarray(inputs["Wfc"], np.float32)[0]
    Wa_l = np.asarray(inputs["Wa_l"], np.float32)
    Wa_r = np.asarray(inputs["Wa_r"], np.float32)
    Wcr_l = np.asarray(inputs["Wcr_l"], np.float32)
    Wcr_r = np.asarray(inputs["Wcr_r"], np.float32)
    u3 = np.stack([Wa_l.T @ w1, Wa_l.T @ w2, Wcr_l.T @ wfc_row],
                  axis=1).astype(fp16)
    v3 = np.stack([Wa_r.T @ w1, Wa_r.T @ w2, Wcr_r.T @ wfc_row],
                  axis=1).astype(fp16)
    ba_l = np.asarray(inputs["ba_l"], np.float32)
    bcr_l = np.asarray(inputs["bcr_l"], np.float32)
    c3 = np.array([[w1 @ ba_l], [w2 @ ba_l], [wfc_row @ bcr_l]],
                  np.float32)
    bfc = np.asarray(inputs["bfc"], np.float32).reshape(1, 1)

    common = {
        # partition-major: sbuf[p, k*U+f] = x[k*128+p, f]
        "xnat": np.ascontiguousarray(
            x.astype(fp16).reshape(KT, 128, U).transpose(1, 0, 2).reshape(
                128, KT * U)),
        "wts": wts,
        "biases": biases,
        "u3": u3,
        "v3": v3,
        "c3": c3,
        "bfc": bfc,
        "iden": np.eye(128, dtype=fp16),
        "ones1": np.ones((1, 1), np.float32),
        "ones128": np.ones((128, 1), np.float32),
    }
    in_maps = []
    for c in range(NCORES):
        sl = slice(c * R, (c + 1) * R)
        m = dict(common)
        # partition-major: sbuf[p, k*R+i] = ATn[k*128+p, own_i]
        m["amat"] = np.ascontiguousarray(
            ATn[:, sl].astype(fp16).reshape(KT, 128, R).transpose(
                1, 0, 2).reshape(128, KT * R))
        m["xt"] = np.ascontiguousarray(x[sl].T.astype(fp16))
        in_maps.append(m)
    return in_maps


def _run(inputs, trace=False):
    from concourse.bass_utils import run_bass_kernel_spmd
    nc = _get_nc()
    in_maps = _host_prep(inputs)
    res = run_bass_kernel_spmd(nc, in_maps, core_ids=list(range(NCORES)),
                               trace=trace)
    edge_actor = np.concatenate(
        [np.asarray(res.results[c]["out"], np.float32) for c in range(NCORES)],
        axis=0).reshape(N * N, 1)
    edge_critic = np.asarray(res.results[0]["crit"], np.float32).reshape(1, 1)
    return (edge_actor, edge_critic), res


def kernel(**inputs):
    outputs, _ = _run(inputs, trace=False)
    return outputs


# revision 14
# speedup vs baseline: 1.2210x; 1.0567x over previous
"""Distributed Trainium2 Bass kernel for the A2C GNN message-passing model.

Strategy (8 NeuronCores, node-row sharding, 512 rows/core):
  - SAGE aggregation as dense-adjacency matmuls on TensorE: the host builds
    AT[j,i] = count(src=j -> dst=i) / max(indeg(i),1) once; each core keeps its
    512-column slice in SBUF (fp16) and computes agg_T[f, own_i] = sum_k
    h_nat[k-chunk] @ AT[k-chunk] (32 accumulating matmuls, f32 PSUM).
  - Feature maps are kept transposed ([feat, node]) so SAGE biases are
    per-partition activation biases; tanh fused into the PSUM->SBUF copy.
  - After layers 1 and 2 the 512 new rows are AllGathered (fp16, 128KB/rank)
    to rebuild the full natural-layout h for the next aggregation.
  - Actor and critic heads share the layer-3 aggregation. Only the projected
    scalars a = Xa@w1, b = Xa@w2, cp = Xc@wfc are AllGathered (f32, 6KB).
  - The N^2 log_softmax factorizes: out[i,j] = a_i + b_j + bfa - LSE with
    LSE = bfa + logsumexp(a) + logsumexp(b), so bfa cancels and
    out[i,j] = b_j + (a_i - La - Lb). Each core writes its 512x4096 f32 block:
    partition-broadcast of the b row + one tensor_scalar add per tile.
  - edge_critic = tanh(mean(cp) + bfc) (the (cp_i+cp_j)/2 mean collapses).

Perf notes (trace-driven):
  - A dummy AllGather with no dependencies runs first so the model-entry CC
    barrier / stream spin-up / cold RDH overlap the HBM load phase instead of
    stalling the first real collective.
  - amat/xnat are staged in DRAM partition-major so each loads as ONE
    dma_start (dispatch on the Sync/Act queues costs ~0.4-2.4us per DMA).
  - The post-AllGather h reload is split per rank-block so layer k+1's
    accumulation matmuls start while later blocks are still in flight.
  - The output is built in 16 [128,1024] tiles: b-row broadcast quarters on
    GpSimd pipeline with DVE adds and output DMAs alternating Sync/Act.
"""

import numpy as np

N = 4096
U = 128
NCORES = 8
R = N // NCORES          # 512 rows per core
KT = N // 128            # 32 contraction chunks
RC = R // 128            # 4 row chunks per core

fp16 = np.float16

_STATE = {}


def _build_nc():
    import concourse.bass as bass
    import concourse.bacc as bacc
    import concourse.mybir as mybir
    import concourse.tile as tile

    f32 = mybir.dt.float32
    f16 = mybir.dt.float16
    AX = mybir.AxisListType.X
    AF = mybir.ActivationFunctionType

    nc = bacc.Bacc("TRN2", target_bir_lowering=False, debug=False,
                   num_devices=NCORES)

    # ---- kernel I/O ----
    amat = nc.dram_tensor("amat", [128, KT * R], f16, kind="ExternalInput")
    xnat = nc.dram_tensor("xnat", [128, KT * U], f16, kind="ExternalInput")
    xt = nc.dram_tensor("xt", [U, R], f16, kind="ExternalInput")
    wts = nc.dram_tensor("wts", [U, 4 * U], f16, kind="ExternalInput")
    biases = nc.dram_tensor("biases", [U, 2], f32, kind="ExternalInput")
    u3 = nc.dram_tensor("u3", [U, 3], f16, kind="ExternalInput")
    v3 = nc.dram_tensor("v3", [U, 3], f16, kind="ExternalInput")
    c3 = nc.dram_tensor("c3", [3, 1], f32, kind="ExternalInput")
    bfc = nc.dram_tensor("bfc", [1, 1], f32, kind="ExternalInput")
    iden = nc.dram_tensor("iden", [128, 128], f16, kind="ExternalInput")
    ones1 = nc.dram_tensor("ones1", [1, 1], f32, kind="ExternalInput")
    ones128 = nc.dram_tensor("ones128", [128, 1], f32, kind="ExternalInput")

    out = nc.dram_tensor("out", [R, N], f32, kind="ExternalOutput")
    crit = nc.dram_tensor("crit", [1, 1], f32, kind="ExternalOutput")

    # ---- collective bounce buffers (internal DRAM) ----
    ag_in = [nc.dram_tensor("ag1_in", [RC, 128, U], f16)]
    ag_out = [nc.dram_tensor("ag1_out", [NCORES, RC, 128, U], f16,
                             addr_space="Shared")]
    ags_in = nc.dram_tensor("ags_in", [128, 12], f16)
    ags_out = nc.dram_tensor("ags_out", [NCORES, 128, 12], f16,
                             addr_space="Shared")
    ag3_in = nc.dram_tensor("ag3_in", [3, R], f32)
    dum_scratch = nc.dram_tensor("dum_scratch", [1, 1], f32)
    ag3_out = nc.dram_tensor("ag3_out", [NCORES, 3, R], f32,
                             addr_space="Shared")
    rgroups = [list(range(NCORES))]

    with tile.TileContext(nc) as tc:
        with tc.tile_pool(name="const", bufs=1) as cst, \
             tc.tile_pool(name="work", bufs=1) as wrk, \
             tc.tile_pool(name="big", bufs=1) as big, \
             tc.tile_pool(name="opool", bufs=4) as opool, \
             tc.tile_pool(name="pag", bufs=1, space="PSUM") as pagp, \
             tc.tile_pool(name="pw", bufs=1, space="PSUM") as pwp, \
             tc.tile_pool(name="pt", bufs=1, space="PSUM") as ptp, \
             tc.tile_pool(name="psm", bufs=1, space="PSUM") as psm:

            # ---- constant loads (few big dispatches, split across queues) --
            xnat_sb = cst.tile([128, KT * U], f16, tag="xnat")
            nc.scalar.dma_start(xnat_sb[:], xnat[:, :])
            amat_sb = cst.tile([128, KT * R], f16, tag="amat")
            nc.sync.dma_start(amat_sb[:, :KT * R // 2], amat[:, :KT * R // 2])
            nc.sync.dma_start(amat_sb[:, KT * R // 2:], amat[:, KT * R // 2:])
            xt_sb = cst.tile([128, R], f16, tag="xt")
            nc.scalar.dma_start(xt_sb[:], xt[:, :])
            wts_sb = cst.tile([128, 4 * U], f16, tag="wts")
            nc.scalar.dma_start(wts_sb[:], wts[:, :])
            bias_sb = cst.tile([128, 2], f32, tag="bias")
            nc.scalar.dma_start(bias_sb[:], biases[:, :])
            u3_sb = cst.tile([128, 3], f16, tag="u3")
            nc.scalar.dma_start(u3_sb[:], u3[:, :])
            v3_sb = cst.tile([128, 3], f16, tag="v3")
            nc.scalar.dma_start(v3_sb[:], v3[:, :])
            c3_sb = cst.tile([3, 1], f32, tag="c3")
            nc.scalar.dma_start(c3_sb[:], c3[:, :])
            bfc_sb = cst.tile([1, 1], f32, tag="bfc")
            nc.scalar.dma_start(bfc_sb[:], bfc[:, :])
            iden_sb = cst.tile([128, 128], f16, tag="iden")
            nc.scalar.dma_start(iden_sb[:], iden[:, :])
            one1_sb = cst.tile([1, 1], f32, tag="one1")
            nc.scalar.dma_start(one1_sb[:], ones1[:, :])
            one128_sb = cst.tile([128, 1], f32, tag="one128")
            nc.scalar.dma_start(one128_sb[:], ones128[:, :])

            hT = [None, None]     # transposed own-columns h

            def sage_agg(lhs_sb):
                """agg_T[f, own_i] accumulated over 32 k-chunk matmuls."""
                pag = pagp.tile([128, R], f32, tag="pag")
                for k in range(KT):
                    nc.tensor.matmul(pag[:], lhs_sb[:, k * U:(k + 1) * U],
                                     amat_sb[:, k * R:(k + 1) * R],
                                     start=(k == 0), stop=(k == KT - 1))
                aggT = wrk.tile([128, R], f16, tag="aggT")
                nc.scalar.copy(aggT[:], pag[:])
                return aggT

            # ================= layers 1 and 2 =================
            hn1 = None
            for l in range(2):
                lhs = xnat_sb if l == 0 else hn1
                rhsT = xt_sb if l == 0 else hT[0]
                aggT = sage_agg(lhs)
                ph = pwp.tile([128, R], f32, tag="ph")
                nc.tensor.matmul(ph[:], wts_sb[:, (2 * l) * U:(2 * l + 1) * U],
                                 aggT[:], start=True, stop=False)
                nc.tensor.matmul(ph[:], wts_sb[:, (2 * l + 1) * U:(2 * l + 2) * U],
                                 rhsT[:], start=False, stop=True)
                hT_new = wrk.tile([128, R], f16, tag=f"hT{l}")
                nc.scalar.activation(hT_new[:], ph[:], AF.Tanh,
                                     bias=bias_sb[:, l:l + 1])
                hT[l] = hT_new
                if l == 0:
                    # transpose own columns back to natural layout and gather
                    pt = ptp.tile([128, R], f16, tag="pt")
                    for c in range(RC):
                        nc.tensor.transpose(pt[:, c * 128:(c + 1) * 128],
                                            hT_new[:, c * 128:(c + 1) * 128],
                                            iden_sb[:])
                    hc = wrk.tile([128, R], f16, tag="hc0")
                    nc.scalar.copy(hc[:], pt[:])
                    nc.sync.dma_start(
                        ag_in[0].ap().rearrange("c p f -> p c f"),
                        hc[:].rearrange("p (c f) -> p c f", c=RC))
                    nc.gpsimd.collective_compute(
                        "AllGather", mybir.AluOpType.bypass,
                        ins=[ag_in[0].ap().opt()], outs=[ag_out[0].ap().opt()],
                        replica_groups=rgroups)
                    hn1 = big.tile([128, KT * U], f16, tag="hnat0")
                    for r in range(NCORES):
                        for h, eng in enumerate((nc.sync, nc.scalar)):
                            o = r * RC * U + h * (RC // 2) * U
                            eng.dma_start(
                                hn1[:, o:o + (RC // 2) * U].rearrange(
                                    "p (k f) -> p k f", k=RC // 2),
                                ag_out[0][r][h * (RC // 2):(h + 1) * (RC // 2)]
                                .rearrange("k p f -> p k f"))

            # ============== scalarized heads ==============
            # a_i = sum_j A^[i,j] s1_j + c1 + v1.h2_i with s_kj = u_k.h2_j;
            # only the 3 s-scalars per node are gathered, not h2 itself.
            hT2 = hT[1]
            pspm = psm.tile([128, 12], f32, tag="pspm")
            for c in range(RC):
                nc.tensor.matmul(pspm[:, c * 3:(c + 1) * 3],
                                 hT2[:, c * 128:(c + 1) * 128], u3_sb[:],
                                 start=True, stop=True)
            pt3 = psm.tile([3, R], f32, tag="pt3")
            nc.tensor.matmul(pt3[:], v3_sb[:], hT2[:], start=True, stop=True)
            t3_sb = wrk.tile([3, R], f32, tag="t3_sb")
            nc.vector.tensor_copy(t3_sb[:], pt3[:])
            s_sb = wrk.tile([128, 12], f16, tag="s_sb")
            nc.vector.tensor_copy(s_sb[:], pspm[:])
            nc.sync.dma_start(ags_in[:, :], s_sb[:])
            # dummy Exp+Ln pre-load both ACT tables before the gathers so
            # the post-gather logsumexp chain runs table-warm; the DRAM write
            # keeps the chain from being dead-code-eliminated or deferred
            dum = wrk.tile([1, 1], f32, tag="dum")
            nc.scalar.activation(dum[:], one1_sb[:], AF.Exp)
            dum2 = wrk.tile([1, 1], f32, tag="dum2")
            nc.scalar.activation(dum2[:], dum[:], AF.Ln)
            nc.scalar.dma_start(dum_scratch[:, :], dum2[:])
            nc.gpsimd.collective_compute(
                "AllGather", mybir.AluOpType.bypass,
                ins=[ags_in.ap().opt()], outs=[ags_out.ap().opt()],
                replica_groups=rgroups)
            s_all = wrk.tile([128, NCORES * 12], f16, tag="s_all")
            nc.gpsimd.dma_start(s_all[:].rearrange("p (r w) -> p r w", r=NCORES),
                                ags_out.ap().rearrange("r p w -> p r w"))
            pagS = psm.tile([3, R], f32, tag="pagS")
            for k in range(KT):
                nc.tensor.matmul(pagS[:], s_all[:, k * 3:(k + 1) * 3],
                                 amat_sb[:, k * R:(k + 1) * R],
                                 start=(k == 0), stop=(k == KT - 1))
            abc_own = wrk.tile([3, R], f32, tag="abc_own")
            nc.vector.tensor_tensor(abc_own[:], pagS[:], t3_sb[:],
                                    op=mybir.AluOpType.add)
            nc.vector.tensor_scalar_add(abc_own[:], abc_own[:], c3_sb[:, 0:1])
            nc.sync.dma_start(ag3_in[:, :], abc_own[:])
            nc.gpsimd.collective_compute(
                "AllGather", mybir.AluOpType.bypass,
                ins=[ag3_in.ap().opt()], outs=[ag3_out.ap().opt()],
                replica_groups=rgroups)

            # ---- global logsumexp of a and b (parallel [128, 32] layout) ----
            abcpm = wrk.tile([128, 3 * 32], f32, tag="abcpm")
            for t in range(3):
                nc.sync.dma_start(abcpm[:, t * 32:(t + 1) * 32],
                                  ag3_out[:, t, :])
            # bb[p, j] = b[j]: broadcast-read DMA straight from ag3_out,
            # quartered so the first output tiles start early; overlaps the
            # LSE chain and never touches GpSimd (no post-collective drain).
            CQ = N // 4
            bb = big.tile([128, N], f32, tag="bb")
            for r in range(NCORES):
                nc.gpsimd.dma_start(bb[:, r * R:(r + 1) * R],
                                    ag3_out[r, 1, :].partition_broadcast(128))

            # logsumexp pieces, ACT ops grouped by activation table
            negms, ess, emps = [], [], []
            for t in range(2):
                v = abcpm[:, t * 32:(t + 1) * 32]
                negm = wrk.tile([128, 1], f32, tag=f"negm{t}")
                nc.vector.reduce_max(negm[:], v, axis=AX, negate=True)
                negms.append(negm)
            for t in range(2):
                v = abcpm[:, t * 32:(t + 1) * 32]
                e = wrk.tile([128, 32], f32, tag=f"e{t}")
                es = wrk.tile([128, 1], f32, tag=f"es{t}")
                nc.scalar.activation(e[:], v, AF.Exp, bias=negms[t][:, 0:1],
                                     accum_out=es[:, 0:1])
                ess.append(es)
            for t in range(2):
                emp = wrk.tile([128, 1], f32, tag=f"emp{t}")
                nc.scalar.activation(emp[:], negms[t][:], AF.Exp, scale=-1.0)
                emps.append(emp)
            Ls = []
            for t in range(2):
                # total = sum_p es_p * exp(m_p) as a PE dot product
                ptot = psm.tile([1, 1], f32, tag="small")
                nc.tensor.matmul(ptot[:], ess[t][:], emps[t][:],
                                 start=True, stop=True)
                L = wrk.tile([1, 1], f32, tag=f"L{t}")
                nc.scalar.activation(L[:], ptot[:], AF.Ln)
                Ls.append(L)

            negL2 = wrk.tile([1, 1], f32, tag="negL2")
            nc.vector.tensor_tensor(negL2[:], Ls[0][:], Ls[1][:],
                                    op=mybir.AluOpType.add)
            nc.scalar.mul(negL2[:], negL2[:], -1.0)

            # alpha[i] = a_own[i] - La - Lb, moved to partition axis
            arow = wrk.tile([1, R], f32, tag="arow")
            nc.vector.tensor_scalar_add(arow[:], abc_own[0:1, :],
                                        negL2[0:1, 0:1])
            pa = psm.tile([128, RC], f32, tag="small")
            for c in range(RC):
                nc.tensor.matmul(pa[:, c:c + 1],
                                 arow[0:1, c * 128:(c + 1) * 128],
                                 one1_sb[:], start=True, stop=True)
            alpha = wrk.tile([128, RC], f32, tag="alpha")
            nc.vector.tensor_copy(alpha[:], pa[:])

            # ---- the big output: out[i, j] = b[j] + alpha[i] ----
            CH = N // 2
            di = 0
            for q in range(2):
                for c in range(RC):
                    ob = opool.tile([128, CH], f32, tag="ob")
                    nc.vector.tensor_scalar_add(ob[:], bb[:, q * CH:(q + 1) * CH],
                                                alpha[:, c:c + 1])
                    eng = nc.sync if di % 2 == 0 else nc.scalar
                    di += 1
                    eng.dma_start(
                        out[c * 128:(c + 1) * 128, q * CH:(q + 1) * CH],
                        ob[:])

            # critic: tanh(sum(cp)/N + bfc) — off the critical path
            cps = wrk.tile([128, 1], f32, tag="cps")
            nc.vector.reduce_sum(cps[:], abcpm[:, 64:96], axis=AX)
            pct = psm.tile([1, 1], f32, tag="small")
            nc.tensor.matmul(pct[:], one128_sb[:], cps[:], start=True, stop=True)
            crit_sb = wrk.tile([1, 1], f32, tag="crit_sb")
            nc.scalar.activation(crit_sb[:], pct[:], AF.Tanh,
                                 scale=1.0 / N, bias=bfc_sb[0:1, 0:1])
            nc.sync.dma_start(crit[:, :], crit_sb[:])

    nc.compile()
    return nc


def _get_nc():
    if "nc" not in _STATE:
        import concourse.bass as bass  # noqa: F401
        _STATE["nc"] = _build_nc()
    return _STATE["nc"]


def _host_prep(inputs):
    x = np.asarray(inputs["x"], np.float32)
    ei = np.asarray(inputs["edge_index"])
    src = ei[0].astype(np.int64)
    dst = ei[1].astype(np.int64)

    AT = np.zeros((N, N), np.float32)
    np.add.at(AT, (src, dst), 1.0)
    deg = np.bincount(dst, minlength=N).astype(np.float32)
    ATn = AT / np.maximum(deg, 1.0)[None, :]

    wts = np.concatenate([
        inputs["Wf_l"].T, inputs["Wf_r"].T,
        inputs["Wcm_l"].T, inputs["Wcm_r"].T,
    ], axis=1).astype(fp16)
    biases = np.stack([
        inputs["bf_l"], inputs["bcm_l"],
    ], axis=1).astype(np.float32)
    Wfa = np.asarray(inputs["Wfa"], np.float32)
    w1, w2 = Wfa[0, :U], Wfa[0, U:]
    wfc_row = np.asarray(inputs["Wfc"], np.float32)[0]
    Wa_l = np.asarray(inputs["Wa_l"], np.float32)
    Wa_r = np.asarray(inputs["Wa_r"], np.float32)
    Wcr_l = np.asarray(inputs["Wcr_l"], np.float32)
    Wcr_r = np.asarray(inputs["Wcr_r"], np.float32)
    u3 = np.stack([Wa_l.T @ w1, Wa_l.T @ w2, Wcr_l.T @ wfc_row],
                  axis=1).astype(fp16)
    v3 = np.stack([Wa_r.T @ w1, Wa_r.T @ w2, Wcr_r.T @ wfc_row],
                  axis=1).astype(fp16)
    ba_l = np.asarray(inputs["ba_l"], np.float32)
    bcr_l = np.asarray(inputs["bcr_l"], np.float32)
    c3 = np.array([[w1 @ ba_l], [w2 @ ba_l], [wfc_row @ bcr_l]],
                  np.float32)
    bfc = np.asarray(inputs["bfc"], np.float32).reshape(1, 1)

    common = {
        # partition-major: sbuf[p, k*U+f] = x[k*128+p, f]
        "xnat": np.ascontiguousarray(
            x.astype(fp16).reshape(KT, 128, U).transpose(1, 0, 2).reshape(
                128, KT * U)),
        "wts": wts,
        "biases": biases,
        "u3": u3,
        "v3": v3,
        "c3": c3,
        "bfc": bfc,
        "iden": np.eye(128, dtype=fp16),
        "ones1": np.ones((1, 1), np.float32),
        "ones128": np.ones((128, 1), np.float32),
    }
    in_maps = []
    for c in range(NCORES):
        sl = slice(c * R, (c + 1) * R)
        m = dict(common)
        # partition-major: sbuf[p, k*R+i] = ATn[k*128+p, own_i]
        m["amat"] = np.ascontiguousarray(
            ATn[:, sl].astype(fp16).reshape(KT, 128, R).transpose(
                1, 0, 2).reshape(128, KT * R))
        m["xt"] = np.ascontiguousarray(x[sl].T.astype(fp16))
        in_maps.append(m)
    return in_maps


def _run(inputs, trace=False):
    from concourse.bass_utils import run_bass_kernel_spmd
    nc = _get_nc()
    in_maps = _host_prep(inputs)
    res = run_bass_kernel_spmd(nc, in_maps, core_ids=list(range(NCORES)),
                               trace=trace)
    edge_actor = np.concatenate(
        [np.asarray(res.results[c]["out"], np.float32) for c in range(NCORES)],
        axis=0).reshape(N * N, 1)
    edge_critic = np.asarray(res.results[0]["crit"], np.float32).reshape(1, 1)
    return (edge_actor, edge_critic), res


def kernel(**inputs):
    outputs, _ = _run(inputs, trace=False)
    return outputs
